# revision 18
# baseline (speedup 1.0000x reference)
"""SNN 5-layer conv net (nn_Net_55405078118821) for 8 Trainium2 cores.

Data-parallel over batch: each core processes 4 of 32 batch elements.

Per-core dataflow (all intermediates stay in SBUF):
  - x slice DMA'd in with layout [24p = (h%8)*3+c, bank(h//8) x t x w]
  - per layer: conv expressed as banded matmuls. Spatial rows are packed
    8-per-SBUF-bank along partitions: input bank q holds rows 8q..8q+7 as
    partitions (row%8)*Cin + cin. An output block of 8 rows needs input
    rows 8q..8q+10 -> two matmuls (K=128 from bank q, K<=48 from bank q+1),
    x 4 column taps (dj) handled by shifting the rhs free-dim offset,
    all 8 accumulating into one PSUM bank of [M=8*Cout, 8t x Wout].
  - LIF scan per timestep over 4-bank groups:
      u = dv + v          (DVE, reads PSUM)
      v' = (u < vth) * u  (DVE scalar_tensor_tensor)
      s = (u >= vth)      (Pool tensor_scalar) -> written into next layer's
                          input layout (spikes of block q land in bank q).
  - layer-5 spikes DMA'd to DRAM; host computes the spatial mean.
"""

import numpy as np

import concourse.bass as bass
import concourse.bacc as bacc
import concourse.mybir as mybir
from concourse.tile import TileContext
from concourse.bass_utils import run_bass_kernel_spmd

N_CORES = 8
B_FULL, T = 32, 16
B_LOC = B_FULL // N_CORES
F32 = mybir.dt.float32
F32R = mybir.dt.float32r
MM_DT = F32R  # matmul operand view dtype (f32r = full-rate fp32 path)

# (Cin, Cout, Hin, Win) per layer; Hout = Hin-3, Wout = Win-3
LAYER_SHAPES = [(3, 16, 64, 64), (16, 16, 61, 61), (16, 16, 58, 58),
                (16, 16, 55, 55), (16, 6, 52, 52)]


class LayerCfg:
    def __init__(self, idx, cin, cout, hin, win):
        self.idx = idx
        self.cin, self.cout, self.hin, self.win = cin, cout, hin, win
        self.hout, self.wout = hin - 3, win - 3
        self.nbk_out = (self.hout + 7) // 8          # output banks
        self.nbk_in = (hin + 7) // 8                 # input banks
        self.mfull = 8 * cout                        # full-block M
        # L1 stacks x-hi/x-lo along K (mult=2); spike layers are exact
        self.kmult = 2 if idx == 0 else 1
        self.k1max = 8 * cin * self.kmult
        self.k2max = 3 * cin * self.kmult
        self.p = 8 * cout if cout * 8 <= 128 else 128  # scan partition count
        # per output block q: rows produced R, K sizes
        self.blocks = []
        for q in range(self.nbk_out):
            r = min(8, self.hout - 8 * q)
            r1 = min(8, hin - 8 * q)
            r2 = max(0, r - 5)
            self.blocks.append((q, r, r1 * cin * self.kmult,
                                r2 * cin * self.kmult))
        # groups of <=4 blocks (each group = <=4 PSUM banks in flight)
        self.groups = [self.blocks[i:i + 4] for i in range(0, len(self.blocks), 4)]


CFGS = [LayerCfg(i, *s) for i, s in enumerate(LAYER_SHAPES)]
L5 = CFGS[-1]
SOUT_FREE = L5.nbk_out * T * L5.wout  # 7*16*49 = 5488


def _pack_A(w):
    """Banded stationary matrix for in-bank rows. w: [Cout,Cin,4,4].
    A[(rm*Cin+ci), dj*Mf + rho*Cout+co] = w[co,ci,rm-rho,dj] for 0<=rm-rho<=3."""
    cout, cin = w.shape[0], w.shape[1]
    mf = 8 * cout
    a = np.zeros((8 * cin, 4 * mf), np.float32)
    for dj in range(4):
        for rm in range(8):
            for rho in range(max(0, rm - 3), rm + 1):
                di = rm - rho
                # [ci, co] block
                a[rm * cin:(rm + 1) * cin, dj * mf + rho * cout: dj * mf + (rho + 1) * cout] = \
                    w[:, :, di, dj].T
    return a


def _pack_B(w):
    """Stationary matrix for the 3 spill rows from bank q+1 (rows 8q+8..8q+10).
    B[(r8*Cin+ci), dj*Mf + rho*Cout+co] = w[co,ci,r8+8-rho,dj] for 0<=r8+8-rho<=3."""
    cout, cin = w.shape[0], w.shape[1]
    mf = 8 * cout
    b = np.zeros((3 * cin, 4 * mf), np.float32)
    for dj in range(4):
        for r8 in range(3):
            for rho in range(max(0, r8 + 5), 8):
                di = r8 + 8 - rho
                if 0 <= di <= 3:
                    b[r8 * cin:(r8 + 1) * cin, dj * mf + rho * cout: dj * mf + (rho + 1) * cout] = \
                        w[:, :, di, dj].T
    return b


def _trunc11(a):
    """Truncate fp32 mantissa to 11 fractional bits (exact under f32r)."""
    u = np.ascontiguousarray(a, np.float32).view(np.uint32)
    return (u & np.uint32(0xFFFFF000)).view(np.float32)


def _expand_rows(a, pattern):
    """Interleave rows of matrices per pattern: out[i*n + j] = pattern[j]-th
    source's row i (source None -> zeros)."""
    n = len(pattern)
    out = np.zeros((a[0].shape[0] * n, a[0].shape[1]), np.float32)
    for j, srcm in enumerate(pattern):
        if srcm is not None:
            out[j::n] = srcm
    return out


def _pack_layer_weights(w, first):
    """Returns (wA, wB) with free dim (sp,dj)-major: offset (sp*4+dj)*Mf.
    sp=0 pass: W-hi (L1: applied to both x-hi and x-lo rows);
    sp=1 pass: W-lo (L1: applied to x-hi rows only)."""
    w = np.asarray(w, np.float32)
    w0, w1 = _trunc11(w), None
    w1 = (w - w0).astype(np.float32)
    a0, a1 = _pack_A(w0), _pack_A(w1)
    b0, b1 = _pack_B(w0), _pack_B(w1)
    if first:
        # K rows interleaved (x-hi, x-lo): sp0 -> [W0; W0], sp1 -> [W1; 0]
        a_sp0 = _expand_rows([a0], [a0, a0])
        a_sp1 = _expand_rows([a1], [a1, None])
        b_sp0 = _expand_rows([b0], [b0, b0])
        b_sp1 = _expand_rows([b1], [b1, None])
    else:
        a_sp0, a_sp1, b_sp0, b_sp1 = a0, a1, b0, b1
    return (np.concatenate([a_sp0, a_sp1], axis=1),
            np.concatenate([b_sp0, b_sp1], axis=1))


def _pack_vth(vths):
    """[128, 5] per-partition thresholds; partition = rho*Cout + co."""
    vb = np.full((128, 5), 1e30, np.float32)
    for li, cfg in enumerate(CFGS):
        v = vths[li].reshape(-1)  # [Cout]
        for p in range(8 * cfg.cout):
            vb[p, li] = v[p % cfg.cout]
    return vb


_PROGRAM_CACHE = {}


def _build_program():
    if "nc" in _PROGRAM_CACHE:
        return _PROGRAM_CACHE["nc"]
    nc = bacc.Bacc("TRN2", target_bir_lowering=False, debug=False)

    # x pre-arranged on host: [b, p=(h%8)*3+c, (h//8) x t x w]
    x_d = nc.dram_tensor("xr", [B_LOC, 48, 8 * T * 64], F32,
                         kind="ExternalInput").ap()
    wa_d, wb_d = [], []
    for li, cfg in enumerate(CFGS):
        wa_d.append(nc.dram_tensor(f"wA{li + 1}", [cfg.k1max, 8 * cfg.mfull], F32,
                                   kind="ExternalInput").ap())
        wb_d.append(nc.dram_tensor(f"wB{li + 1}", [cfg.k2max, 8 * cfg.mfull], F32,
                                   kind="ExternalInput").ap())
    vth_d = nc.dram_tensor("vthb", [128, 5], F32, kind="ExternalInput").ap()
    sout_d = nc.dram_tensor("sout", [B_LOC, 48, SOUT_FREE], F32,
                            kind="ExternalOutput").ap()

    with TileContext(nc) as tc:
        with (
            tc.tile_pool(name="wts", bufs=1) as wts,
            tc.tile_pool(name="xin", bufs=2) as xpool,
            tc.tile_pool(name="spk", bufs=1) as spool,
            tc.tile_pool(name="scan", bufs=4) as upool,
            tc.tile_pool(name="vst", bufs=2) as vpool,
            tc.tile_pool(name="psum", bufs=2, space="PSUM") as ppool,
        ):
            # --- load constants ---
            wa_t, wb_t = [], []
            for li, cfg in enumerate(CFGS):
                ta = wts.tile([cfg.k1max, 8 * cfg.mfull], MM_DT, tag=f"wa{li}")
                nc.gpsimd.dma_start(out=ta[:, :], in_=wa_d[li])
                wa_t.append(ta)
                tb = wts.tile([cfg.k2max, 8 * cfg.mfull], MM_DT, tag=f"wb{li}")
                nc.gpsimd.dma_start(out=tb[:, :], in_=wb_d[li])
                wb_t.append(tb)
            vth_t = wts.tile([128, 5], F32, tag="vth")
            nc.sync.dma_start(out=vth_t[:, :], in_=vth_d)
            zero_t = wts.tile([128, 244], F32, tag="zero")
            nc.any.memset(zero_t[:, :], 0.0)

            for b in range(B_LOC):
                # --- x DMA: [24p=(h%8)*3+c, q, t, w] ---
                x_t = xpool.tile([48, 8 * T * 64], MM_DT, tag="x")
                x_v = x_t[:, :].rearrange("p (q t w) -> p q w t", q=8, t=T)
                nc.gpsimd.dma_start(out=x_t[:, :], in_=x_d[b])

                prev_tile, prev_view = None, None
                for li, cfg in enumerate(CFGS):
                    cin, cout = cfg.cin, cfg.cout
                    mf, wo, wi = cfg.mfull, cfg.wout, cfg.win
                    # spike-output tile in next layer's input layout
                    s_t = spool.tile([cfg.p, cfg.nbk_out * T * wo], MM_DT,
                                     tag=f"s{li % 2}", name=f"s_b{b}l{li}")
                    s_v = s_t[:, :].rearrange("p (q t w) -> p q t w",
                                              q=cfg.nbk_out, t=T)
                    s_mm = s_t[:, :].rearrange("p (q t w) -> p q w t",
                                               q=cfg.nbk_out, t=T)
                    in_view = x_v if li == 0 else prev_view

                    # v-state per group, persists across both t-halves
                    v_ts = []
                    for g in range(len(cfg.groups)):
                        v_ts.append(vpool.tile([cfg.p, 4 * wo], F32, tag="v",
                                               name=f"v_b{b}l{li}g{g}"))

                    for h in range(2):          # t-halves
                        for g, blocks in enumerate(cfg.groups):
                            nbk = len(blocks)
                            ps = ppool.tile([128, 2048], F32, tag="ps")
                            ps_v = ps[:, :].rearrange("p (k n) -> p k n", n=512)
                            ps_jt = ps[:, :].rearrange("p (k j t) -> p k j t",
                                                       k=4, j=64, t=8)
                            # --- conv matmuls: fill nbk banks ---
                            for bi, (q, r, k1, k2) in enumerate(blocks):
                                # full-M matmul: rows beyond r are unused
                                # (but initialized) — scan reads [0:p]
                                n = 8 * wo
                                out_ap = ps_v[0:cfg.p, bi, 0:n]
                                n_mm = 2 * 4 * (2 if k2 > 0 else 1)
                                mm = 0
                                for sp in range(2):
                                    for dj in range(4):
                                        c0 = (sp * 4 + dj) * mf
                                        lhs = wa_t[li][0:k1, c0:c0 + mf]
                                        rhs = in_view[0:k1, q, dj:dj + wo,
                                                      h * 8:(h + 1) * 8]
                                        nc.tensor.matmul(
                                            out_ap, lhs, rhs,
                                            start=(mm == 0), stop=(mm == n_mm - 1))
                                        mm += 1
                                        if k2 > 0:
                                            lhs2 = wb_t[li][0:k2, c0:c0 + mf]
                                            rhs2 = in_view[0:k2, q + 1,
                                                           dj:dj + wo,
                                                           h * 8:(h + 1) * 8]
                                            nc.tensor.matmul(
                                                out_ap, lhs2, rhs2,
                                                start=False, stop=(mm == n_mm - 1))
                                            mm += 1
                            # --- LIF scan over the 8 timesteps of this half ---
                            p = cfg.p
                            q0 = g * 4
                            vth_ap = vth_t[0:p, li:li + 1]
                            v_v = v_ts[g][:, :].rearrange(
                                "p (k w) -> p k w", w=wo)[0:p, 0:nbk, :]
                            for t in range(8):
                                tt = h * 8 + t
                                dv = ps_jt[0:p, 0:nbk, 0:wo, t]
                                u_t = upool.tile([cfg.p, 4 * wo], F32, tag="u")
                                u_v = u_t[:, :].rearrange(
                                    "p (k w) -> p k w", w=wo)[0:p, 0:nbk, :]
                                v_in = zero_t[:, :].rearrange(
                                    "p (k w) -> p k w", w=61)[0:p, 0:nbk, 0:wo] \
                                    if tt == 0 else v_v
                                nc.vector.tensor_tensor(
                                    out=u_v, in0=dv, in1=v_in,
                                    op=mybir.AluOpType.add)
                                nc.vector.scalar_tensor_tensor(
                                    out=v_v, in0=u_v, scalar=vth_ap, in1=u_v,
                                    op0=mybir.AluOpType.is_lt,
                                    op1=mybir.AluOpType.mult)
                                s_out = s_v[0:p, q0:q0 + nbk, tt, :]
                                nc.gpsimd.tensor_scalar(
                                    out=s_out, in0=u_v, scalar1=vth_ap,
                                    scalar2=None, op0=mybir.AluOpType.is_ge)
                    prev_tile, prev_view = s_t, s_mm

                # --- emit layer-5 spikes ---
                nc.gpsimd.dma_start(out=sout_d[b], in_=prev_tile[0:48, :])

    nc.compile()
    _PROGRAM_CACHE["nc"] = nc
    return nc


def _arrange_x(x):
    """[b,T,3,64,64] -> [b, 48 = ((h%8)*3+c)*2+sp, (h//8) x t x w],
    sp=0 hi (trunc11), sp=1 lo (remainder)."""
    bl = x.shape[0]
    x = np.ascontiguousarray(x, np.float32)
    x0 = _trunc11(x)
    x1 = x - x0
    xs = np.stack([x0, x1], axis=-1)            # b t c h w sp
    xs = xs.reshape(bl, T, 3, 8, 8, 64, 2)      # b t c q hm w sp
    xs = xs.transpose(0, 4, 2, 6, 3, 1, 5)      # b hm c sp q t w
    return np.ascontiguousarray(xs.reshape(bl, 48, 8 * T * 64), dtype=np.float32)


def _host_inputs(inputs):
    ws = [inputs[f"w{i + 1}"] for i in range(5)]
    vths = [inputs[f"vth{i + 1}"] for i in range(5)]
    m = {}
    for li in range(5):
        wa, wb = _pack_layer_weights(ws[li], first=(li == 0))
        m[f"wA{li + 1}"] = wa
        m[f"wB{li + 1}"] = wb
    m["vthb"] = _pack_vth([np.asarray(v, np.float32) for v in vths])
    return m


def decode_sout(sout):
    """[B_LOC, 48, SOUT_FREE] -> [B_LOC, T, 6] spike means."""
    a = sout.reshape(B_LOC, 8, 6, L5.nbk_out, T, L5.wout)
    # valid output rows: h = 8q + rho < 49
    rho = np.arange(8)[:, None]
    qq = np.arange(L5.nbk_out)[None, :]
    mask = (8 * qq + rho) < L5.hout                     # [rho, q]
    a = a.transpose(0, 4, 2, 1, 3, 5)                   # [b, t, c, rho, q, j]
    vals = a[:, :, :, mask, :]                          # [b, t, c, 49, 49]
    return vals.mean(axis=(3, 4)).astype(np.float32)


def run_spmd(inputs, **kw):
    nc = _build_program()
    x = np.asarray(inputs["x"], np.float32)
    const = _host_inputs(inputs)
    in_maps = []
    for c in range(N_CORES):
        m = dict(const)
        m["xr"] = _arrange_x(x[c * B_LOC:(c + 1) * B_LOC])
        in_maps.append(m)
    return run_bass_kernel_spmd(nc, in_maps, list(range(N_CORES)), **kw)


def kernel(**inputs):
    res = run_spmd(inputs)
    outs = [decode_sout(r["sout"]) for r in res.results]
    return np.concatenate(outs, axis=0)
